# revision 1
# baseline (speedup 1.0000x reference)
"""Trainium2 Bass kernel for BasicRecurrentEntityEncoder.

Strategy: data-parallel over the batch (paragraph) dim: 8 cores x 8 paragraphs.
Per core, everything lives in a "columns" layout [d=128 partitions, n=160 free]
where n = b_local*20 + k (8 paragraphs x 20 entity slots).

Phase A (overlapped with recurrence): embedding lookup via one indirect-DMA
gather of 10240 token rows (masked tokens point at an appended zero row),
position-mask multiply on DVE, and per-128-token-block block-diagonal matmuls
that accumulate per-sentence sums into PSUM -> encoded_T [128, 512 sentences].

Phase B: 64 serial recurrence steps. All reductions over d and all broadcasts
across d are TensorE matmuls (ones / identity tricks). Transcendentals use only
the natural_log_exp ACT table set (no table switches):
  sigmoid(z) = 1/(1+exp(-z))      (ACT Exp + DVE add1 + DVE reciprocal_approx)
  rsqrt(ss)  = exp(-0.5*ln(ss+eps)) (ACT Ln + ACT Exp)
Sentence masking folds into the gate: masked columns get a -1e5 bias on the
gate preactivation (clamped to -60), so gate ~ 2e-27 -> h unchanged (h rows are
always exactly 0 or unit-norm, so the re-normalization is a no-op on them).
"""

import numpy as np

B, S, L, K, D, VOC = 64, 64, 20, 20, 128, 50000
NCORES = 8
BL = B // NCORES          # paragraphs per core = 8
N = BL * K                # recurrence columns = 160
TOK = BL * S * L          # gathered tokens per core = 10240
TPB = 120                 # tokens per block (6 whole sentences -> no PSUM accum)
NBLK = (TOK + TPB - 1) // TPB   # 86 blocks (last one padded with zero rows)
CHUNKS = [11, 11, 11, 11, 11, 11, 10, 10]   # gather/encode groups (blocks)
ZROW = VOC                # index of appended all-zero embedding row
GATE_BIAS = -1.0e5        # mask bias on gate preactivation
CLAMP = 40.0              # |z| clamp before exp (exp(40)=2.4e17 < 2^64 for ACT Ln)

_NC_CACHE = {}


def _build_nc():
    import concourse.bass as bass
    import concourse.tile as tile
    from concourse import mybir

    f32 = mybir.dt.float32
    i32 = mybir.dt.int32
    AF = mybir.ActivationFunctionType
    OP = mybir.AluOpType

    nc = bass.Bass()

    CW = 1104  # ident|ones|zeros|U|V|W|keysT|omap|posw|eps
    d_emb = nc.declare_dram_parameter("emb", [VOC + 1, D], f32, isOutput=False)
    d_idx = nc.declare_dram_parameter("idx", [TPB, NBLK], i32, isOutput=False)
    d_mb = nc.declare_dram_parameter("mb", [1, S * N], f32, isOutput=False)
    d_cst = nc.declare_dram_parameter("cst", [128, CW], f32, isOutput=False)
    d_h0 = nc.declare_dram_parameter("h0", [D, N], f32, isOutput=False)
    d_out = nc.declare_dram_parameter("out", [D, N], f32, isOutput=True)
    d_enc = nc.declare_dram_parameter("enc_out", [D, S * BL], f32, isOutput=True)

    from contextlib import ExitStack
    with ExitStack() as ctx:
        tc = ctx.enter_context(tile.TileContext(nc))
        singles = ctx.enter_context(tc.tile_pool(name="singles", bufs=1))
        wpool = ctx.enter_context(tc.tile_pool(name="wtile", bufs=3))
        step_sb = ctx.enter_context(tc.tile_pool(name="step_sb", bufs=2))
        hpool = ctx.enter_context(tc.tile_pool(name="hpool", bufs=2))
        p_enc = ctx.enter_context(tc.tile_pool(name="p_enc", bufs=1, space="PSUM"))
        p_hk = ctx.enter_context(tc.tile_pool(name="p_hk", bufs=1, space="PSUM"))
        p_g = ctx.enter_context(tc.tile_pool(name="p_g", bufs=1, space="PSUM"))
        p_ht = ctx.enter_context(tc.tile_pool(name="p_ht", bufs=1, space="PSUM"))
        p_ss = ctx.enter_context(tc.tile_pool(name="p_ss", bufs=1, space="PSUM"))

        # ---- constants into SBUF (ONE DMA so consumers wait on one lane) ----
        idx_sb = singles.tile([TPB, NBLK], i32)
        nc.sync.dma_start(out=idx_sb[:, :], in_=d_idx[:, :])
        mb_sb = singles.tile([1, S * N], f32)
        nc.sync.dma_start(out=mb_sb[:, :], in_=d_mb[:, :])
        cst_sb = singles.tile([128, CW], f32)
        nc.sync.dma_start(out=cst_sb[:, :], in_=d_cst[:, :])
        id_sb = cst_sb[:, 0:128]
        ones_sb = cst_sb[:, 128:256]
        zrow_sb = cst_sb[:, 256:384]
        U_sb = cst_sb[:, 384:512]
        V_sb = cst_sb[:, 512:640]
        W_sb = cst_sb[:, 640:768]
        keysT_sb = cst_sb[:, 768:928]
        omap_sb = cst_sb[0:TPB, 928:934]
        posw_sb = cst_sb[0:TPB, 944:1072]
        eps_sb = cst_sb[:, 1103:1104]

        # warmups: one tiny op per (engine, DMA-lane) so real instructions
        # never need more than one semaphore wait
        warm = singles.tile([1, 4], f32)
        nc.vector.tensor_copy(out=warm[0:1, 0:1], in_=cst_sb[0:1, 0:1])
        nc.vector.tensor_copy(out=warm[0:1, 1:2], in_=mb_sb[0:1, 0:1])
        nc.scalar.copy(out=warm[0:1, 2:3], in_=cst_sb[0:1, 0:1])

        G_sb = singles.tile([128, NBLK * D], f32)  # gathered token rows
        enc_sb = singles.tile([128, S * BL], f32)  # encoded sentences (d, s*8+b)
        psum_enc = p_enc.tile([128, S * BL], f32)

        # ---- Phase A: gather + position-weighted sentence sums ----
        # Each 120-token block holds 6 whole sentences, so each sentence's sum
        # is produced by exactly ONE start=True matmul (no PSUM accumulation).
        j0 = 0
        for nb in CHUNKS:
            for j in range(j0, j0 + nb):
                nc.gpsimd.indirect_dma_start(
                    out=G_sb[0:TPB, j * D:(j + 1) * D],
                    out_offset=None,
                    in_=d_emb[:, :],
                    in_offset=bass.IndirectOffsetOnAxis(ap=idx_sb[:, j:j + 1], axis=0),
                )
                w = min(6, S * BL - 6 * j)
                wt = wpool.tile([TPB, D], f32)
                nc.vector.tensor_mul(
                    wt[:, :],
                    G_sb[0:TPB, j * D:(j + 1) * D],
                    posw_sb,
                )
                nc.tensor.matmul(
                    out=psum_enc[:, 6 * j:6 * j + w],
                    lhsT=wt[:, :],
                    rhs=omap_sb[:, 0:w],
                    start=True, stop=True,
                )
            c0, c1 = 6 * j0, min(6 * (j0 + nb), S * BL)
            nc.scalar.copy(out=enc_sb[:, c0:c1], in_=psum_enc[:, c0:c1])
            j0 += nb

        # ---- Phase B: 64 recurrence steps ----
        h_prev = hpool.tile([D, N], f32, tag="h")
        nc.sync.dma_start(out=h_prev[:, :], in_=d_h0[:, :])

        for s in range(S):
            e8 = enc_sb[:, s * BL:(s + 1) * BL]
            e_rep = bass.AP(tensor=e8.tensor, offset=e8.offset,
                            ap=[e8.ap[0], e8.ap[1], [0, K]])
            e_mat = step_sb.tile([D, N], f32, tag="e_mat")
            nc.vector.tensor_copy(
                out=e_mat[:, :].rearrange("p (a b) -> p a b", a=BL),
                in_=e_rep,
            )

            # gate preactivation: colsum(e * (h + keys)) + mask bias
            psum_hk = p_hk.tile([D, N], f32, tag="hk")
            nc.tensor.matmul(out=psum_hk[:, :], lhsT=id_sb, rhs=h_prev[:, :],
                             start=True, stop=False)
            nc.tensor.matmul(out=psum_hk[:, :], lhsT=id_sb, rhs=keysT_sb[:, :],
                             start=False, stop=True)
            tmp1 = step_sb.tile([D, N], f32, tag="tmp1")
            nc.vector.tensor_mul(tmp1[:, :], psum_hk[:, :], e_mat[:, :])
            psum_g = p_g.tile([D, N], f32, tag="g")
            nc.tensor.matmul(out=psum_g[:, :], lhsT=ones_sb, rhs=tmp1[:, :],
                             start=True, stop=False)
            nc.tensor.matmul(out=psum_g[:, :], lhsT=ones_sb[0:1, :],
                             rhs=mb_sb[0:1, s * N:(s + 1) * N], start=False, stop=True)
            zc = step_sb.tile([D, N], f32, tag="zc")
            nc.vector.tensor_scalar(out=zc[:, :], in0=psum_g[:, :],
                                    scalar1=-CLAMP, scalar2=CLAMP,
                                    op0=OP.max, op1=OP.min)
            Ex = step_sb.tile([D, N], f32, tag="Ex")
            nc.scalar.activation(Ex[:, :], zc[:, :], AF.Exp, bias=0.0, scale=-1.0)
            Pd = step_sb.tile([D, N], f32, tag="Pd")
            nc.vector.tensor_scalar_add(Pd[:, :], Ex[:, :], 1.0)
            Lp = step_sb.tile([D, N], f32, tag="Lp")
            nc.scalar.activation(Lp[:, :], Pd[:, :], AF.Ln)
            Rg = step_sb.tile([D, N], f32, tag="Rg")
            nc.scalar.activation(Rg[:, :], Lp[:, :], AF.Exp, bias=0.0, scale=-1.0)

            # h_tilda = relu(hU + kV + eW), transposed into [d, n]
            psum_ht = p_ht.tile([D, N], f32, tag="ht")
            nc.tensor.matmul(out=psum_ht[:, :], lhsT=U_sb[:, :], rhs=h_prev[:, :],
                             start=True, stop=False)
            nc.tensor.matmul(out=psum_ht[:, :], lhsT=V_sb[:, :], rhs=keysT_sb[:, :],
                             start=False, stop=False)
            nc.tensor.matmul(out=psum_ht[:, :], lhsT=W_sb[:, :], rhs=e_mat[:, :],
                             start=False, stop=True)
            HT = step_sb.tile([D, N], f32, tag="HT")
            nc.scalar.activation(HT[:, :], psum_ht[:, :], AF.Relu)

            Tg = step_sb.tile([D, N], f32, tag="Tg")
            nc.vector.tensor_mul(Tg[:, :], Rg[:, :], HT[:, :])
            yv = step_sb.tile([D, N], f32, tag="yv")
            nc.vector.tensor_add(yv[:, :], h_prev[:, :], Tg[:, :])

            SQ = step_sb.tile([D, N], f32, tag="SQ")
            nc.scalar.activation(SQ[:, :], yv[:, :], AF.Square)
            psum_ss = p_ss.tile([D, N], f32, tag="ss")
            nc.tensor.matmul(out=psum_ss[:, :], lhsT=ones_sb, rhs=SQ[:, :],
                             start=True, stop=True)
            Lg = step_sb.tile([D, N], f32, tag="Lg")
            nc.scalar.activation(Lg[:, :], psum_ss[:, :], AF.Ln, bias=eps_sb)
            RS = step_sb.tile([D, N], f32, tag="RS")
            nc.scalar.activation(RS[:, :], Lg[:, :], AF.Exp, bias=0.0, scale=-0.5)

            h_new = hpool.tile([D, N], f32, tag="h")
            nc.vector.tensor_mul(h_new[:, :], yv[:, :], RS[:, :])
            h_prev = h_new

        nc.sync.dma_start(out=d_out[:, :], in_=h_prev[:, :])
        nc.sync.dma_start(out=d_enc[:, :], in_=enc_sb[:, :])

    return nc


def _legalize_waits(bir_json: bytes) -> bytes:
    """Walrus codegen allows at most ONE sem-wait per instruction; Tile's sem
    assigner emits several. Hoist all but the last wait onto EventSemaphore
    carrier instructions inserted just before the offender (same engine, so
    in-order execution preserves semantics exactly)."""
    import orjson
    bir = orjson.loads(bir_json)
    n_new = 0
    for fn in bir.get("functions", []):
        for bb in fn.get("blocks", []):
            out = []
            for inst in bb.get("instructions", []):
                si = inst.get("sync_info") or {}
                ow = si.get("on_wait") or []
                if len(ow) > 1:
                    for w in ow[:-1]:
                        n_new += 1
                        out.append({
                            "debug": inst.get("debug", 0),
                            "engine": inst["engine"],
                            "ins": [], "outs": [],
                            "name": f"waitfix_{n_new}_{inst.get('name','')}",
                            "opcode": "EventSemaphore",
                            "sync_info": {"on_update": [], "on_wait": [w]},
                        })
                    si["on_wait"] = [ow[-1]]
                    inst["sync_info"] = si
                out.append(inst)
            bb["instructions"] = out
    return orjson.dumps(bir)


def _install_compile_hook():
    import concourse.bass2jax as b2j
    if getattr(b2j, "_waitfix_installed", False):
        return
    orig = b2j.compile_bir_kernel

    def patched(bir_json, tmpdir, neff_name="file.neff"):
        return orig(_legalize_waits(bir_json), tmpdir, neff_name)

    b2j.compile_bir_kernel = patched
    b2j._waitfix_installed = True


def get_nc():
    if "nc" not in _NC_CACHE:
        _NC_CACHE["nc"] = _build_nc()
    return _NC_CACHE["nc"]


def make_inputs_for_core(c, prgrph, prgrph_mask, keys, emb_aug, pos_mask, U, V, W,
                         posw, omap, cst):
    f32 = np.float32
    bsl = slice(c * BL, (c + 1) * BL)
    pr = np.asarray(prgrph[bsl]).astype(np.int64)      # [BL, S, L]
    mk = np.asarray(prgrph_mask[bsl]).astype(bool)     # [BL, S, L]
    ky = np.asarray(keys[bsl], dtype=f32)              # [BL, K, D]

    # token order: t = ((s*BL + b)*L + l)  (s-major); block j = t//TPB, row t%TPB
    idx_flat = np.where(mk, pr, ZROW).transpose(1, 0, 2).reshape(-1).astype(np.int32)
    idx_pad = np.full(NBLK * TPB, ZROW, dtype=np.int32)
    idx_pad[:TOK] = idx_flat
    idx2d = np.ascontiguousarray(idx_pad.reshape(NBLK, TPB).T)  # [TPB, NBLK]

    keysT = np.ascontiguousarray(ky.transpose(2, 0, 1).reshape(D, N))

    m_s = mk[:, :, 0].astype(f32)                       # [BL, S]
    mb = (m_s.T - 1.0) * (-GATE_BIAS)                   # [S, BL]: 0 or GATE_BIAS
    mb = np.repeat(mb[:, :, None], K, axis=2).reshape(1, S * N).astype(f32)

    cst_c = cst.copy()
    cst_c[:, 384:512] = np.asarray(U, dtype=f32)
    cst_c[:, 512:640] = np.asarray(V, dtype=f32)
    cst_c[:, 640:768] = np.asarray(W, dtype=f32)
    cst_c[:, 768:928] = keysT
    return {
        "cst": np.ascontiguousarray(cst_c),
        "h0": np.zeros((D, N), dtype=f32),
        "emb": emb_aug,
        "idx": idx2d,
        "mb": np.ascontiguousarray(mb, dtype=f32),
    }


def make_shared_consts(embedding_matrix, pos_mask):
    f32 = np.float32
    emb_aug = np.vstack([np.asarray(embedding_matrix, dtype=f32),
                         np.zeros((1, D), dtype=f32)])
    emb_aug = np.ascontiguousarray(emb_aug)

    pm = np.asarray(pos_mask, dtype=f32)
    posw = np.empty((TPB, D), dtype=f32)
    for p in range(TPB):
        posw[p] = pm[p % L]

    omap = np.zeros((TPB, 6), dtype=f32)
    for p in range(TPB):
        omap[p, p // L] = 1.0

    cst = np.zeros((128, 1104), dtype=f32)
    cst[:, 0:128] = np.eye(128, dtype=f32)
    cst[:, 128:256] = 1.0
    cst[0:TPB, 928:934] = omap
    cst[0:TPB, 944:1072] = posw
    cst[:, 1103] = 1e-12
    return emb_aug, posw, omap, cst


def kernel(prgrph, prgrph_mask, keys, embedding_matrix, pos_mask, U, V, W):
    from concourse.bass_utils import run_bass_kernel_spmd
    _install_compile_hook()

    emb_aug, posw, omap, cst = make_shared_consts(embedding_matrix, pos_mask)
    in_maps = [
        make_inputs_for_core(c, prgrph, prgrph_mask, keys, emb_aug, pos_mask,
                             U, V, W, posw, omap, cst)
        for c in range(NCORES)
    ]
    nc = get_nc()
    res = run_bass_kernel_spmd(nc, in_maps, core_ids=list(range(NCORES)))
    outs = []
    for c in range(NCORES):
        o = np.asarray(res.results[c]["out"])        # [D, N]
        outs.append(o.T.reshape(BL, K, D))
    return np.concatenate(outs, axis=0).astype(np.float32)

